# revision 27
# baseline (speedup 1.0000x reference)
"""DDSL simplex-FT Bass kernel for Trainium2 (8 NeuronCores).

Math: for triangles (j=2) with vertices P[e,v,:] (from V[E]), densities D,
output spectrum F over the 256x129 rfft2 grid:

  sig_v(e,f)  = 2*pi*(kx*Px_v + ky*Py_v)
  d01=sig0-sig1, d12=sig1-sig2, d20=sig2-sig0,  Q = d01*d12*d20
  tmp_re = -(d12*cos(sig0)+d20*cos(sig1)+d01*cos(sig2))/Q
  tmp_im = +(d12*sin(sig0)+d20*sin(sig1)+d01*sin(sig2))/Q
  F_raw  = sum_e CD_e * tmp;  F = -(256^2)*F_raw  (+ DC override)

Sharding: frequency rows split 8 ways (32 kx rows x 132 padded ky cols per
core = 33 chunks of 128 freqs on partitions); duplicate elements are merged
on the host (D aggregated), the survivor count padded to n_pad (136 here)
on the free dim. No collective needed: each core owns its rows; the host
concatenates.

Per chunk-pair on device:
  - PE builds u_v = kx*Px+ky*Py, u_v+0.25, d01, d12, CD*2pi*d_pair planes as
    K=6/7 bf16 matmuls over 3-way bf16 splits (products exact, fp32 accum).
  - DVE (custom fused ops): FRAC (arg = 2pi*(u - round(u)) via the +1.5*2^23
    magic round, 6 planes in one pass), QR3 (-Q = d01*d12*(d01+d12), zero-
    guarded), 51-ULP reciprocal, G_v = (CD*d_pair)*(-1/Q), then one fused
    multiply+free-dim-reduce per (chunk, re/im) that performs the element
    AND vertex sums in a single pass (free dim spans all 3 vertex planes).
  - ACT: Sin on reduced args in [-pi, pi] (cos via the u+0.25 planes), and
    one PSUM->SBUF copy so QR3 has at most one PSUM operand.
Host: gather V[E], dedupe, exact split tables, final sign/scale, DC bin,
unshard. Measured vs fp32 jax reference: l2 relative error ~4.1e-5.
"""

import math
import numpy as np
import ml_dtypes

N_CORES = 8
N_ELEM = 256
RES0, RES1 = 256, 129
KYPAD = 132  # 32*132 = 4224 = 33*128
ROWS_PER_CORE = 32
CHUNKS = (ROWS_PER_CORE * KYPAD) // 128  # 33
MAGIC = float(np.float32(1.5 * 2**23))
TWO_PI = 2 * math.pi

_compiled = {}


def _split3(v):
    """3-way bf16 split of fp32/64 values: v ~= h+m+l with exact bf16 parts."""
    v32 = np.asarray(v, np.float32)
    h = v32.astype(ml_dtypes.bfloat16)
    r = (v32 - h.astype(np.float32)).astype(np.float32)
    m = r.astype(ml_dtypes.bfloat16)
    l = (r - m.astype(np.float32)).astype(ml_dtypes.bfloat16)
    return h, m, l


def _register_ops():
    import concourse.dve_ops as dve_ops_mod
    from concourse.dve_ops import DveOp, OPS
    from concourse.dve_spec import (
        Spec,
        Src0,
        Src1,
        C0,
        C1,
        One,
        Zero,
        eq,
        select,
        lower as dve_lower,
        _has_src1 as has_src1,
    )
    from concourse.dve_uop import DveOpSpec
    import operator

    def register_op(name, spec, subdim=False):
        existing = {op.name: op for op in OPS}
        if name in existing:
            return existing[name]
        opcode = dve_ops_mod._CUSTOM_DVE_ROW_BASE + len(OPS)
        assert opcode < 0x20
        dve_ops_mod._SUB_OPCODE_FOR_NAME[name] = opcode
        shas = {}
        for ver in ("v3",):
            uops = dve_lower(spec, ver=ver)
            shas[ver] = DveOpSpec(
                name=name, opcode=opcode, uops=uops, rd1_en=has_src1(spec)
            ).sha(ver)
        op = DveOp(name, spec, subdim=subdim, uops_sha=shas)
        OPS.append(op)
        dve_ops_mod.CUSTOM_DVE_SPECS[name] = spec
        return op

    frac = register_op("FRAC_SCALED", Spec(body=(Src0 - ((Src0 + C0) - C0)) * C1))
    _q = Src0 * Src1 * (Src0 + Src1)
    qr3 = register_op("QR3_GUARD", Spec(body=select(eq(_q, Zero), One, _q)))
    mulacc = register_op(
        "MUL_ACC_SEED", Spec(body=Src0 * Src1, accum=operator.add, accum_init=C0)
    )
    return frac, qr3, mulacc


def _build_program(n_pad):
    import concourse.bacc as bacc
    import concourse.mybir as mybir
    from concourse.tile import TileContext

    FRAC, QR3, MUL_ACC = _register_ops()

    f32 = mybir.dt.float32
    bf16 = mybir.dt.bfloat16
    nc = bacc.Bacc("TRN2", target_bir_lowering=False)

    lhs_d = nc.dram_tensor("lhs7", [7, CHUNKS * 128], bf16, kind="ExternalInput")
    rhsu_d = nc.dram_tensor("rhsu", [6, 3 * n_pad], bf16, kind="ExternalInput")
    rhsuc_d = nc.dram_tensor("rhsuc", [7, 3 * n_pad], bf16, kind="ExternalInput")
    rhsd_d = nc.dram_tensor("rhsd", [6, 2 * n_pad], bf16, kind="ExternalInput")
    rhsg_d = nc.dram_tensor("rhsg", [6, 3 * n_pad], bf16, kind="ExternalInput")
    fout_d = nc.dram_tensor("fout", [128, 2 * CHUNKS], f32, kind="ExternalOutput")

    E = n_pad
    Sin = mybir.ActivationFunctionType.Sin

    with TileContext(nc) as tc:
        with (
            tc.tile_pool(name="const", bufs=1) as cpool,
            tc.tile_pool(name="work", bufs=4) as pool,
            tc.tile_pool(name="cols", bufs=8) as colpool,
            tc.tile_pool(name="psum", bufs=1, space="PSUM") as psp,
        ):
            lhs = cpool.tile([7, CHUNKS * 128], bf16)
            rhsu = cpool.tile([6, 3 * E], bf16)
            rhsuc = cpool.tile([7, 3 * E], bf16)
            rhsd = cpool.tile([6, 2 * E], bf16)
            rhsg = cpool.tile([6, 3 * E], bf16)
            fout = cpool.tile([128, 2 * CHUNKS], f32)
            # matmul outputs must not straddle PSUM bank boundaries:
            # either the 3-slot arena fits one bank, or slots are 1KB-aligned
            assert 12 * E <= 2048 or E == 256, f"bad n_pad {E}"
            nc.sync.dma_start(lhs[:], lhs_d[:])
            nc.sync.dma_start(rhsu[:], rhsu_d[:])
            nc.sync.dma_start(rhsuc[:], rhsuc_d[:])
            nc.sync.dma_start(rhsd[:], rhsd_d[:])
            nc.sync.dma_start(rhsg[:], rhsg_d[:])

            # process chunks in pairs: PE matmuls stay per-chunk, but
            # FRAC/Sin/QR/recip/G run once per pair over strided rank-3 APs.
            # PSUM pair arenas are padded to 512-col halves so no matmul
            # output crosses a bank boundary.
            HB = 512  # psum half stride (cols)
            pairs = [
                [2 * p, 2 * p + 1] if 2 * p + 1 < CHUNKS else [2 * p]
                for p in range((CHUNKS + 1) // 2)
            ]
            for chunks in pairs:
                T = len(chunks)
                EB = 3 * E  # one chunk's block of 3 vertex planes

                # uu: [us(h0)|us(h1)|uc(h0)|uc(h1)] in 512-col halves (4 banks)
                uu = psp.tile([128, 4 * HB], f32, tag="uu")
                dd = psp.tile([128, 2 * HB], f32, tag="dd")
                gg = psp.tile([128, 2 * HB], f32, tag="gg")

                mm = nc.tensor.matmul
                for h, c in enumerate(chunks):
                    l6 = lhs[0:6, c * 128 : (c + 1) * 128]
                    l7 = lhs[:, c * 128 : (c + 1) * 128]
                    b = h * HB
                    # group by stationary operand: all l6 matmuls, then l7
                    for v in range(3):
                        mm(uu[:, b + v * E : b + (v + 1) * E], l6,
                           rhsu[:, v * E : (v + 1) * E], start=True, stop=True)
                    mm(dd[:, b : b + E], l6, rhsd[:, 0:E], start=True, stop=True)
                    mm(dd[:, b + E : b + 2 * E], l6, rhsd[:, E : 2 * E],
                       start=True, stop=True)
                    for v in range(3):
                        mm(gg[:, b + v * E : b + (v + 1) * E], l6,
                           rhsg[:, v * E : (v + 1) * E], start=True, stop=True)
                    for v in range(3):
                        mm(uu[:, 2 * HB + b + v * E : 2 * HB + b + (v + 1) * E],
                           l7, rhsuc[:, v * E : (v + 1) * E],
                           start=True, stop=True)

                def psum_blocks(ap, off, width):
                    """(128, nblk, width) view of used cols of a PSUM arena
                    whose data sits in T-strided 512-col halves."""
                    stride = HB if T == 2 else 2 * HB
                    return ap.rearrange("p (t x) -> p t x", x=stride)[
                        :, :, off : off + width
                    ]

                def blocks(ap, off, width, stride):
                    """(128, nblk, width) view of a compact tile."""
                    return ap.rearrange("p (t x) -> p t x", x=stride)[
                        :, :, off : off + width
                    ]

                # FRAC: arg = 2*pi*(u - round(u)); sin blocks then cos blocks
                arg = pool.tile([128, 2 * T * EB], f32, tag="arg")
                cd = nc.vector._custom_dve
                cd(FRAC, out=blocks(arg[:], 0, EB, EB),
                   in0=psum_blocks(uu[:], 0, EB), s0=MAGIC, s1=TWO_PI)

                # trig: [s(h0)|s(h1)|c(h0)|c(h1)] chunk blocks of [v0|v1|v2]
                tr = pool.tile([128, 2 * T * EB], f32, tag="tr")
                nc.scalar.activation(tr[:], arg[:], Sin)

                d12s = pool.tile([128, T * E], f32, tag="d12s")
                nc.scalar.activation(
                    blocks(d12s[:], 0, E, E), psum_blocks(dd[:], E, E),
                    mybir.ActivationFunctionType.Copy,
                )
                mQ = pool.tile([128, T * E], f32, tag="mQ")
                cd(QR3, out=blocks(mQ[:], 0, E, E), in0=blocks(d12s[:], 0, E, E),
                   in1=psum_blocks(dd[:], 0, E))
                # 51-ULP reciprocal is plenty: overall error is dominated by
                # fp32 trig-argument rounding (verified vs fp64 reference)
                R = pool.tile([128, T * E], f32, tag="R")
                nc.vector.reciprocal_approx_fast(out=R[:], in_=mQ[:])

                # Gt layout: chunk-major [h][v] so the v-sum can fuse into
                # the accumulation pass
                Gt = pool.tile([128, T * EB], f32, tag="Gt")
                for v in range(3):
                    nc.vector.tensor_mul(
                        blocks(Gt[:], v * E, E, EB),
                        psum_blocks(gg[:], v * E, E),
                        blocks(R[:], 0, E, E),
                    )

                # one fused multiply+reduce per (chunk, component): the free
                # dim spans all 3 vertex planes, so accum does the v-sum too
                scr = pool.tile([128, EB], f32, tag="scr")
                for h, c in enumerate(chunks):
                    g = Gt[:, h * EB : (h + 1) * EB]
                    cd(MUL_ACC, out=scr[:], in0=g,
                       in1=tr[:, (T + h) * EB : (T + h + 1) * EB], s0=0.0,
                       accum_out=fout[:, 2 * c : 2 * c + 1])
                    cd(MUL_ACC, out=scr[:], in0=g,
                       in1=tr[:, h * EB : (h + 1) * EB], s0=0.0,
                       accum_out=fout[:, 2 * c + 1 : 2 * c + 2])

            nc.sync.dma_start(fout_d[:], fout[:])

    nc.compile()
    return nc


def _host_prep_group(P, Dagg, n_pad):
    """Build per-core input maps for one padded element group."""
    n_eff = P.shape[0]
    # pad with copies of element 0 carrying zero density (zero contribution)
    if n_pad > n_eff:
        P = np.concatenate([P, np.repeat(P[:1], n_pad - n_eff, axis=0)], axis=0)
        Dagg = np.concatenate(
            [Dagg, np.zeros((n_pad - n_eff, Dagg.shape[1]))], axis=0
        )
    ne = n_pad

    # CD = 2 * area * D via Cayley-Menger (matches reference up to fp rounding)
    D2 = ((P[:, :, None, :] - P[:, None, :, :]) ** 2).sum(-1)
    B = np.ones((ne, 4, 4))
    B[:, 0, 0] = 0.0
    B[:, 1:, 1:] = D2
    vol2 = (-1.0) / 4.0 * np.linalg.det(B) / 4.0  # ((-1)^3)/(2^2)/(2!^2)*det
    content = np.sqrt(np.clip(vol2, 0.0, None))
    CD = 2.0 * content[:, None] * Dagg  # (ne, n_ch=1)
    cd = CD[:, 0]  # n_ch == 1

    Px = P[:, :, 0]  # (ne, 3)
    Py = P[:, :, 1]
    dPx = Px - np.roll(Px, -1, axis=1)  # [d01, d12, d20] coefficients
    dPy = Py - np.roll(Py, -1, axis=1)

    def stack6(ax, ay):
        """rows [axh, axm, axl, ayh, aym, ayl] as bf16 (256 cols)."""
        xh, xm, xl = _split3(ax)
        yh, ym, yl = _split3(ay)
        return np.stack([xh, xm, xl, yh, ym, yl]).astype(ml_dtypes.bfloat16)

    rhsu = np.concatenate([stack6(Px[:, v], Py[:, v]) for v in range(3)], axis=1)
    quarter = np.full((1, ne), 0.25, ml_dtypes.bfloat16)
    rhsuc = np.concatenate(
        [
            np.concatenate([stack6(Px[:, v], Py[:, v]), quarter], axis=0)
            for v in range(3)
        ],
        axis=1,
    )
    rhsd = np.concatenate(
        [stack6(TWO_PI * dPx[:, k], TWO_PI * dPy[:, k]) for k in (0, 1)], axis=1
    )
    # gg_v pairs: c0<->d12, c1<->d20, c2<->d01
    pair = [1, 2, 0]
    rhsg = np.concatenate(
        [
            stack6(TWO_PI * cd * dPx[:, pair[v]], TWO_PI * cd * dPy[:, pair[v]])
            for v in range(3)
        ],
        axis=1,
    )

    kxv = np.fft.fftfreq(RES0, d=1.0 / RES0)  # row -> freq value
    in_maps = []
    for r in range(N_CORES):
        q = np.arange(CHUNKS * 128)
        lr = q // KYPAD
        kyi = q % KYPAD
        kxrow = kxv[32 * r + lr]
        lhs = np.zeros((7, CHUNKS * 128), np.float32)
        lhs[0:3] = kxrow
        lhs[3:6] = kyi
        lhs[6] = 1.0
        in_maps.append(
            {
                "lhs7": lhs.astype(ml_dtypes.bfloat16),
                "rhsu": rhsu,
                "rhsuc": rhsuc,
                "rhsd": rhsd,
                "rhsg": rhsg,
            }
        )
    return in_maps, float(np.sum(cd))


# largest element count whose 3-plane PSUM arena fits one 512-col half
_MAX_GROUP = 168


def kernel(V, E, D, _want_trace=False):
    from concourse.bass_utils import run_bass_kernel_spmd

    V = np.asarray(V, np.float32)
    E = np.asarray(E)
    D = np.asarray(D, np.float32)

    # identical elements (same vertex-index rows) contribute identical
    # spectra scaled by their D -> deduplicate and aggregate D
    Eu, inv = np.unique(E, axis=0, return_inverse=True)
    Dagg = np.zeros((Eu.shape[0], D.shape[1]), np.float64)
    np.add.at(Dagg, inv.reshape(-1), D.astype(np.float64))
    n_eff = Eu.shape[0]
    P = V[Eu].astype(np.float64)  # (n_eff, 3, 2)

    # split into groups small enough for the PSUM layout; partial spectra
    # are linear in elements, so group results just add
    n_groups = -(-n_eff // _MAX_GROUP)
    per = -(-n_eff // n_groups)
    n_pad = max(8, -(-per // 8) * 8)
    if n_pad not in _compiled:
        _compiled[n_pad] = _build_program(n_pad)
    nc = _compiled[n_pad]

    fo_sum = [np.zeros((128, 2 * CHUNKS), np.float64) for _ in range(N_CORES)]
    cd_total = 0.0
    res = None
    for g in range(n_groups):
        sl = slice(g * per, min((g + 1) * per, n_eff))
        in_maps, cd_sum = _host_prep_group(P[sl], Dagg[sl], n_pad)
        cd_total += cd_sum
        res = run_bass_kernel_spmd(
            nc, in_maps, core_ids=list(range(N_CORES)), trace=_want_trace
        )
        for r in range(N_CORES):
            fo_sum[r] += res.results[r]["fout"]

    F = np.zeros((RES0, RES1, 1, 2), np.float32)
    for r in range(N_CORES):
        fo = fo_sum[r].astype(np.float32)  # (128, 2*CHUNKS)
        re_raw = fo[:, 0::2].T.reshape(-1)  # (33*128,) chunk-major
        im_raw = fo[:, 1::2].T.reshape(-1)
        re = re_raw.reshape(ROWS_PER_CORE, KYPAD)[:, :RES1]
        im = im_raw.reshape(ROWS_PER_CORE, KYPAD)[:, :RES1]
        F[32 * r : 32 * r + 32, :, 0, 0] = -65536.0 * re
        F[32 * r : 32 * r + 32, :, 0, 1] = 65536.0 * im
    F[0, 0, 0, :] = np.float32(32768.0 * cd_total)
    if _want_trace:
        return F, res
    return F


# revision 28
# speedup vs baseline: 1.0346x; 1.0346x over previous
"""DDSL simplex-FT Bass kernel for Trainium2 (8 NeuronCores).

Math: for triangles (j=2) with vertices P[e,v,:] (from V[E]), densities D,
output spectrum F over the 256x129 rfft2 grid:

  sig_v(e,f)  = 2*pi*(kx*Px_v + ky*Py_v)
  d01=sig0-sig1, d12=sig1-sig2, d20=sig2-sig0,  Q = d01*d12*d20
  tmp_re = -(d12*cos(sig0)+d20*cos(sig1)+d01*cos(sig2))/Q
  tmp_im = +(d12*sin(sig0)+d20*sin(sig1)+d01*sin(sig2))/Q
  F_raw  = sum_e CD_e * tmp;  F = -(256^2)*F_raw  (+ DC override)

Sharding: frequency rows split 8 ways (32 kx rows x 132 padded ky cols per
core = 33 chunks of 128 freqs on partitions); duplicate elements are merged
on the host (D aggregated), the survivor count padded to n_pad (136 here)
on the free dim. No collective needed: each core owns its rows; the host
concatenates.

Per chunk-pair on device:
  - PE builds u_v = kx*Px+ky*Py, u_v+0.25, d01, d12, CD*2pi*d_pair planes as
    K=6/7 bf16 matmuls over 3-way bf16 splits (products exact, fp32 accum).
  - DVE (custom fused ops): FRAC (arg = 2pi*(u - round(u)) via the +1.5*2^23
    magic round, 6 planes in one pass), QR3 (-Q = d01*d12*(d01+d12), zero-
    guarded), 51-ULP reciprocal, G_v = (CD*d_pair)*(-1/Q), then one fused
    multiply+free-dim-reduce per (chunk, re/im) that performs the element
    AND vertex sums in a single pass (free dim spans all 3 vertex planes).
  - ACT: Sin on reduced args in [-pi, pi] (cos via the u+0.25 planes), and
    one PSUM->SBUF copy so QR3 has at most one PSUM operand.
Host: gather V[E], dedupe, exact split tables, final sign/scale, DC bin,
unshard. Measured vs fp32 jax reference: l2 relative error ~4.1e-5.
"""

import math
import numpy as np
import ml_dtypes

N_CORES = 8
N_ELEM = 256
RES0, RES1 = 256, 129
KYPAD = 132  # 32*132 = 4224 = 33*128
ROWS_PER_CORE = 32
CHUNKS = (ROWS_PER_CORE * KYPAD) // 128  # 33
MAGIC = float(np.float32(1.5 * 2**23))
TWO_PI = 2 * math.pi

_compiled = {}


def _split3(v):
    """3-way bf16 split of fp32/64 values: v ~= h+m+l with exact bf16 parts."""
    v32 = np.asarray(v, np.float32)
    h = v32.astype(ml_dtypes.bfloat16)
    r = (v32 - h.astype(np.float32)).astype(np.float32)
    m = r.astype(ml_dtypes.bfloat16)
    l = (r - m.astype(np.float32)).astype(ml_dtypes.bfloat16)
    return h, m, l


def _register_ops():
    import concourse.dve_ops as dve_ops_mod
    from concourse.dve_ops import DveOp, OPS
    from concourse.dve_spec import (
        Spec,
        Src0,
        Src1,
        C0,
        C1,
        One,
        Zero,
        eq,
        select,
        lower as dve_lower,
        _has_src1 as has_src1,
    )
    from concourse.dve_uop import DveOpSpec
    import operator

    def register_op(name, spec, subdim=False):
        existing = {op.name: op for op in OPS}
        if name in existing:
            return existing[name]
        opcode = dve_ops_mod._CUSTOM_DVE_ROW_BASE + len(OPS)
        assert opcode < 0x20
        dve_ops_mod._SUB_OPCODE_FOR_NAME[name] = opcode
        shas = {}
        for ver in ("v3",):
            uops = dve_lower(spec, ver=ver)
            shas[ver] = DveOpSpec(
                name=name, opcode=opcode, uops=uops, rd1_en=has_src1(spec)
            ).sha(ver)
        op = DveOp(name, spec, subdim=subdim, uops_sha=shas)
        OPS.append(op)
        dve_ops_mod.CUSTOM_DVE_SPECS[name] = spec
        return op

    frac = register_op("FRAC_SCALED", Spec(body=(Src0 - ((Src0 + C0) - C0)) * C1))
    _q = Src0 * Src1 * (Src0 + Src1)
    qr3 = register_op("QR3_GUARD", Spec(body=select(eq(_q, Zero), One, _q)))
    mulacc = register_op(
        "MUL_ACC_SEED", Spec(body=Src0 * Src1, accum=operator.add, accum_init=C0)
    )
    return frac, qr3, mulacc


def _build_program(n_pad):
    import concourse.bacc as bacc
    import concourse.mybir as mybir
    from concourse.tile import TileContext

    FRAC, QR3, MUL_ACC = _register_ops()

    f32 = mybir.dt.float32
    bf16 = mybir.dt.bfloat16
    nc = bacc.Bacc("TRN2", target_bir_lowering=False)

    lhs_d = nc.dram_tensor("lhs7", [7, CHUNKS * 128], bf16, kind="ExternalInput")
    rhsu_d = nc.dram_tensor("rhsu", [6, 3 * n_pad], bf16, kind="ExternalInput")
    rhsuc_d = nc.dram_tensor("rhsuc", [7, 3 * n_pad], bf16, kind="ExternalInput")
    rhsd_d = nc.dram_tensor("rhsd", [6, 2 * n_pad], bf16, kind="ExternalInput")
    rhsg_d = nc.dram_tensor("rhsg", [6, 3 * n_pad], bf16, kind="ExternalInput")
    fout_d = nc.dram_tensor("fout", [128, 2 * CHUNKS], f32, kind="ExternalOutput")

    E = n_pad
    Sin = mybir.ActivationFunctionType.Sin

    with TileContext(nc) as tc:
        with (
            tc.tile_pool(name="const", bufs=1) as cpool,
            tc.tile_pool(name="work", bufs=4) as pool,
            tc.tile_pool(name="cols", bufs=8) as colpool,
            tc.tile_pool(name="psum", bufs=1, space="PSUM") as psp,
        ):
            lhs = cpool.tile([7, CHUNKS * 128], bf16)
            rhsu = cpool.tile([6, 3 * E], bf16)
            rhsuc = cpool.tile([7, 3 * E], bf16)
            rhsd = cpool.tile([6, 2 * E], bf16)
            rhsg = cpool.tile([6, 3 * E], bf16)
            fout = cpool.tile([128, 2 * CHUNKS], f32)
            # matmul outputs must not straddle PSUM bank boundaries:
            # either the 3-slot arena fits one bank, or slots are 1KB-aligned
            assert 12 * E <= 2048 or E == 256, f"bad n_pad {E}"
            nc.sync.dma_start(lhs[:], lhs_d[:])
            nc.sync.dma_start(rhsu[:], rhsu_d[:])
            nc.sync.dma_start(rhsuc[:], rhsuc_d[:])
            nc.sync.dma_start(rhsd[:], rhsd_d[:])
            nc.sync.dma_start(rhsg[:], rhsg_d[:])

            # process chunks in pairs: PE matmuls stay per-chunk, but
            # FRAC/Sin/QR/recip/G run once per pair over strided rank-3 APs.
            # PSUM pair arenas are padded to 512-col halves so no matmul
            # output crosses a bank boundary.
            HB = 512  # psum half stride (cols)
            pairs = [
                [2 * p, 2 * p + 1] if 2 * p + 1 < CHUNKS else [2 * p]
                for p in range((CHUNKS + 1) // 2)
            ]
            for chunks in pairs:
                T = len(chunks)
                EB = 3 * E  # one chunk's block of 3 vertex planes

                # uu: [us(h0)|us(h1)|uc(h0)|uc(h1)] in 512-col halves (4 banks)
                uu = psp.tile([128, 4 * HB], f32, tag="uu")
                dd = psp.tile([128, 2 * HB], f32, tag="dd")
                gg = psp.tile([128, 2 * HB], f32, tag="gg")

                mm = nc.tensor.matmul
                for h, c in enumerate(chunks):
                    l6 = lhs[0:6, c * 128 : (c + 1) * 128]
                    l7 = lhs[:, c * 128 : (c + 1) * 128]
                    b = h * HB
                    # group by stationary operand: all l6 matmuls, then l7
                    for v in range(3):
                        mm(uu[:, b + v * E : b + (v + 1) * E], l6,
                           rhsu[:, v * E : (v + 1) * E], start=True, stop=True)
                    mm(dd[:, b : b + E], l6, rhsd[:, 0:E], start=True, stop=True)
                    mm(dd[:, b + E : b + 2 * E], l6, rhsd[:, E : 2 * E],
                       start=True, stop=True)
                    for v in range(3):
                        mm(gg[:, b + v * E : b + (v + 1) * E], l6,
                           rhsg[:, v * E : (v + 1) * E], start=True, stop=True)
                    for v in range(3):
                        mm(uu[:, 2 * HB + b + v * E : 2 * HB + b + (v + 1) * E],
                           l7, rhsuc[:, v * E : (v + 1) * E],
                           start=True, stop=True)

                def psum_blocks(ap, off, width):
                    """(128, nblk, width) view of used cols of a PSUM arena
                    whose data sits in T-strided 512-col halves."""
                    stride = HB if T == 2 else 2 * HB
                    return ap.rearrange("p (t x) -> p t x", x=stride)[
                        :, :, off : off + width
                    ]

                def blocks(ap, off, width, stride):
                    """(128, nblk, width) view of a compact tile."""
                    return ap.rearrange("p (t x) -> p t x", x=stride)[
                        :, :, off : off + width
                    ]

                # FRAC: arg = 2*pi*(u - round(u)); sin blocks then cos blocks
                arg = pool.tile([128, 2 * T * EB], f32, tag="arg")
                cd = nc.vector._custom_dve
                cd(FRAC, out=blocks(arg[:], 0, EB, EB),
                   in0=psum_blocks(uu[:], 0, EB), s0=MAGIC, s1=TWO_PI)

                # trig: [s(h0)|s(h1)|c(h0)|c(h1)] chunk blocks of [v0|v1|v2]
                tr = pool.tile([128, 2 * T * EB], f32, tag="tr")
                nc.scalar.activation(tr[:], arg[:], Sin)

                d12s = pool.tile([128, T * E], f32, tag="d12s")
                nc.scalar.activation(
                    blocks(d12s[:], 0, E, E), psum_blocks(dd[:], E, E),
                    mybir.ActivationFunctionType.Copy,
                )
                mQ = pool.tile([128, T * E], f32, tag="mQ")
                cd(QR3, out=blocks(mQ[:], 0, E, E), in0=blocks(d12s[:], 0, E, E),
                   in1=psum_blocks(dd[:], 0, E))
                # 51-ULP reciprocal is plenty: overall error is dominated by
                # fp32 trig-argument rounding (verified vs fp64 reference)
                R = pool.tile([128, T * E], f32, tag="R")
                nc.vector.reciprocal_approx_fast(out=R[:], in_=mQ[:])

                # Gt layout: chunk-major [h][v] so the v-sum can fuse into
                # the accumulation pass
                Gt = pool.tile([128, T * EB], f32, tag="Gt")
                for v in range(3):
                    nc.vector.tensor_mul(
                        blocks(Gt[:], v * E, E, EB),
                        psum_blocks(gg[:], v * E, E),
                        blocks(R[:], 0, E, E),
                    )

                # one fused multiply+reduce per (chunk, component): the free
                # dim spans all 3 vertex planes, so accum does the v-sum too
                scr = pool.tile([128, EB], f32, tag="scr")
                for h, c in enumerate(chunks):
                    g = Gt[:, h * EB : (h + 1) * EB]
                    cd(MUL_ACC, out=scr[:], in0=g,
                       in1=tr[:, (T + h) * EB : (T + h + 1) * EB], s0=0.0,
                       accum_out=fout[:, 2 * c : 2 * c + 1])
                    cd(MUL_ACC, out=scr[:], in0=g,
                       in1=tr[:, h * EB : (h + 1) * EB], s0=0.0,
                       accum_out=fout[:, 2 * c + 1 : 2 * c + 2])

            nc.sync.dma_start(fout_d[:], fout[:])

    nc.compile()
    return nc


def _host_prep_group(P, Dagg, n_pad):
    """Build per-core input maps for one padded element group."""
    n_eff = P.shape[0]
    # pad with copies of element 0 carrying zero density (zero contribution)
    if n_pad > n_eff:
        P = np.concatenate([P, np.repeat(P[:1], n_pad - n_eff, axis=0)], axis=0)
        Dagg = np.concatenate(
            [Dagg, np.zeros((n_pad - n_eff, Dagg.shape[1]))], axis=0
        )
    ne = n_pad

    # CD = 2 * area * D via Cayley-Menger (matches reference up to fp rounding)
    D2 = ((P[:, :, None, :] - P[:, None, :, :]) ** 2).sum(-1)
    B = np.ones((ne, 4, 4))
    B[:, 0, 0] = 0.0
    B[:, 1:, 1:] = D2
    vol2 = (-1.0) / 4.0 * np.linalg.det(B) / 4.0  # ((-1)^3)/(2^2)/(2!^2)*det
    content = np.sqrt(np.clip(vol2, 0.0, None))
    CD = 2.0 * content[:, None] * Dagg  # (ne, n_ch=1)
    cd = CD[:, 0]  # n_ch == 1

    Px = P[:, :, 0]  # (ne, 3)
    Py = P[:, :, 1]
    dPx = Px - np.roll(Px, -1, axis=1)  # [d01, d12, d20] coefficients
    dPy = Py - np.roll(Py, -1, axis=1)

    def stack6(ax, ay):
        """rows [axh, axm, axl, ayh, aym, ayl] as bf16 (256 cols)."""
        xh, xm, xl = _split3(ax)
        yh, ym, yl = _split3(ay)
        return np.stack([xh, xm, xl, yh, ym, yl]).astype(ml_dtypes.bfloat16)

    rhsu = np.concatenate([stack6(Px[:, v], Py[:, v]) for v in range(3)], axis=1)
    quarter = np.full((1, ne), 0.25, ml_dtypes.bfloat16)
    rhsuc = np.concatenate(
        [
            np.concatenate([stack6(Px[:, v], Py[:, v]), quarter], axis=0)
            for v in range(3)
        ],
        axis=1,
    )
    rhsd = np.concatenate(
        [stack6(TWO_PI * dPx[:, k], TWO_PI * dPy[:, k]) for k in (0, 1)], axis=1
    )
    # gg_v pairs: c0<->d12, c1<->d20, c2<->d01
    pair = [1, 2, 0]
    rhsg = np.concatenate(
        [
            stack6(TWO_PI * cd * dPx[:, pair[v]], TWO_PI * cd * dPy[:, pair[v]])
            for v in range(3)
        ],
        axis=1,
    )

    kxv = np.fft.fftfreq(RES0, d=1.0 / RES0)  # row -> freq value
    in_maps = []
    for r in range(N_CORES):
        q = np.arange(CHUNKS * 128)
        lr = q // KYPAD
        kyi = q % KYPAD
        kxrow = kxv[32 * r + lr]
        lhs = np.zeros((7, CHUNKS * 128), np.float32)
        lhs[0:3] = kxrow
        lhs[3:6] = kyi
        lhs[6] = 1.0
        in_maps.append(
            {
                "lhs7": lhs.astype(ml_dtypes.bfloat16),
                "rhsu": rhsu,
                "rhsuc": rhsuc,
                "rhsd": rhsd,
                "rhsg": rhsg,
            }
        )
    return in_maps, float(np.sum(cd))


# largest element count whose 3-plane PSUM arena fits one 512-col half
_MAX_GROUP = 170


def kernel(V, E, D, _want_trace=False):
    from concourse.bass_utils import run_bass_kernel_spmd

    V = np.asarray(V, np.float32)
    E = np.asarray(E)
    D = np.asarray(D, np.float32)

    # identical elements (same vertex-index rows) contribute identical
    # spectra scaled by their D -> deduplicate and aggregate D
    Eu, inv = np.unique(E, axis=0, return_inverse=True)
    Dagg = np.zeros((Eu.shape[0], D.shape[1]), np.float64)
    np.add.at(Dagg, inv.reshape(-1), D.astype(np.float64))
    n_eff = Eu.shape[0]
    P = V[Eu].astype(np.float64)  # (n_eff, 3, 2)

    # split into groups small enough for the PSUM layout; partial spectra
    # are linear in elements, so group results just add
    n_groups = -(-n_eff // _MAX_GROUP)
    per = -(-n_eff // n_groups)
    n_pad = max(8, -(-per // 2) * 2)
    if n_pad not in _compiled:
        _compiled[n_pad] = _build_program(n_pad)
    nc = _compiled[n_pad]

    fo_sum = [np.zeros((128, 2 * CHUNKS), np.float64) for _ in range(N_CORES)]
    cd_total = 0.0
    res = None
    for g in range(n_groups):
        sl = slice(g * per, min((g + 1) * per, n_eff))
        in_maps, cd_sum = _host_prep_group(P[sl], Dagg[sl], n_pad)
        cd_total += cd_sum
        res = run_bass_kernel_spmd(
            nc, in_maps, core_ids=list(range(N_CORES)), trace=_want_trace
        )
        for r in range(N_CORES):
            fo_sum[r] += res.results[r]["fout"]

    F = np.zeros((RES0, RES1, 1, 2), np.float32)
    for r in range(N_CORES):
        fo = fo_sum[r].astype(np.float32)  # (128, 2*CHUNKS)
        re_raw = fo[:, 0::2].T.reshape(-1)  # (33*128,) chunk-major
        im_raw = fo[:, 1::2].T.reshape(-1)
        re = re_raw.reshape(ROWS_PER_CORE, KYPAD)[:, :RES1]
        im = im_raw.reshape(ROWS_PER_CORE, KYPAD)[:, :RES1]
        F[32 * r : 32 * r + 32, :, 0, 0] = -65536.0 * re
        F[32 * r : 32 * r + 32, :, 0, 1] = 65536.0 * im
    F[0, 0, 0, :] = np.float32(32768.0 * cd_total)
    if _want_trace:
        return F, res
    return F


# revision 31
# speedup vs baseline: 1.0478x; 1.0127x over previous
"""DDSL simplex-FT Bass kernel for Trainium2 (8 NeuronCores).

Math: for triangles (j=2) with vertices P[e,v,:] (from V[E]), densities D,
output spectrum F over the 256x129 rfft2 grid:

  sig_v(e,f)  = 2*pi*(kx*Px_v + ky*Py_v)
  d01=sig0-sig1, d12=sig1-sig2, d20=sig2-sig0,  Q = d01*d12*d20
  tmp_re = -(d12*cos(sig0)+d20*cos(sig1)+d01*cos(sig2))/Q
  tmp_im = +(d12*sin(sig0)+d20*sin(sig1)+d01*sin(sig2))/Q
  F_raw  = sum_e CD_e * tmp;  F = -(256^2)*F_raw  (+ DC override)

Sharding: frequency rows split 8 ways (32 kx rows x 132 padded ky cols per
core = 33 chunks of 128 freqs on partitions); duplicate elements are merged
on the host (D aggregated), the survivor count padded to n_pad (130 here)
on the free dim. No collective needed: each core owns its rows; the host
concatenates.

Per chunk-pair on device:
  - PE builds u_v = kx*Px+ky*Py, u_v+0.25, d01, d12, CD*2pi*d_pair planes as
    K=6/7 bf16 matmuls over 3-way bf16 splits (products exact, fp32 accum).
  - DVE (custom fused ops): FRAC (arg = 2pi*(u - round(u)) via the +1.5*2^23
    magic round, 6 planes in one pass), QR3 (-Q = d01*d12*(d01+d12), zero-
    guarded), 51-ULP reciprocal, G_v = (CD*d_pair)*(-1/Q), then one fused
    multiply+free-dim-reduce per (chunk, re/im) that performs the element
    AND vertex sums in a single pass (free dim spans all 3 vertex planes).
  - ACT: Sin on reduced args in [-pi, pi] (cos via the u+0.25 planes), and
    one PSUM->SBUF copy so QR3 has at most one PSUM operand.
Host: gather V[E], dedupe, exact split tables, final sign/scale, DC bin,
unshard. Measured vs fp32 jax reference: l2 relative error ~4.1e-5.
"""

import math
import numpy as np
import ml_dtypes

N_CORES = 8
N_ELEM = 256
RES0, RES1 = 256, 129
KYPAD = 132  # 32*132 = 4224 = 33*128
ROWS_PER_CORE = 32
CHUNKS = (ROWS_PER_CORE * KYPAD) // 128  # 33
MAGIC = float(np.float32(1.5 * 2**23))
TWO_PI = 2 * math.pi

_compiled = {}


def _split3(v):
    """3-way bf16 split of fp32/64 values: v ~= h+m+l with exact bf16 parts."""
    v32 = np.asarray(v, np.float32)
    h = v32.astype(ml_dtypes.bfloat16)
    r = (v32 - h.astype(np.float32)).astype(np.float32)
    m = r.astype(ml_dtypes.bfloat16)
    l = (r - m.astype(np.float32)).astype(ml_dtypes.bfloat16)
    return h, m, l


def _register_ops():
    import concourse.dve_ops as dve_ops_mod
    from concourse.dve_ops import DveOp, OPS
    from concourse.dve_spec import (
        Spec,
        Src0,
        Src1,
        C0,
        C1,
        One,
        Zero,
        eq,
        select,
        lower as dve_lower,
        _has_src1 as has_src1,
    )
    from concourse.dve_uop import DveOpSpec
    import operator

    def register_op(name, spec, subdim=False):
        existing = {op.name: op for op in OPS}
        if name in existing:
            return existing[name]
        opcode = dve_ops_mod._CUSTOM_DVE_ROW_BASE + len(OPS)
        assert opcode < 0x20
        dve_ops_mod._SUB_OPCODE_FOR_NAME[name] = opcode
        shas = {}
        for ver in ("v3",):
            uops = dve_lower(spec, ver=ver)
            shas[ver] = DveOpSpec(
                name=name, opcode=opcode, uops=uops, rd1_en=has_src1(spec)
            ).sha(ver)
        op = DveOp(name, spec, subdim=subdim, uops_sha=shas)
        OPS.append(op)
        dve_ops_mod.CUSTOM_DVE_SPECS[name] = spec
        return op

    frac = register_op("FRAC_SCALED", Spec(body=(Src0 - ((Src0 + C0) - C0)) * C1))
    _q = Src0 * Src1 * (Src0 + Src1)
    qr3 = register_op("QR3_GUARD", Spec(body=select(eq(_q, Zero), One, _q)))
    mulacc = register_op(
        "MUL_ACC_SEED", Spec(body=Src0 * Src1, accum=operator.add, accum_init=C0)
    )
    negadd = register_op("NEG_ADD", Spec(body=Zero - (Src0 + Src1)))
    return frac, qr3, mulacc, negadd


def _build_program(n_pad):
    import concourse.bacc as bacc
    import concourse.mybir as mybir
    from concourse.tile import TileContext

    FRAC, QR3, MUL_ACC, NEG_ADD = _register_ops()

    f32 = mybir.dt.float32
    bf16 = mybir.dt.bfloat16
    nc = bacc.Bacc("TRN2", target_bir_lowering=False)

    lhs_d = nc.dram_tensor("lhs7", [7, CHUNKS * 128], bf16, kind="ExternalInput")
    rhsu_d = nc.dram_tensor("rhsu", [6, 3 * n_pad], bf16, kind="ExternalInput")
    rhsuc_d = nc.dram_tensor("rhsuc", [7, 3 * n_pad], bf16, kind="ExternalInput")
    rhsd_d = nc.dram_tensor("rhsd", [6, 2 * n_pad], bf16, kind="ExternalInput")
    rhsg_d = nc.dram_tensor("rhsg", [6, 3 * n_pad], bf16, kind="ExternalInput")
    fout_d = nc.dram_tensor("fout", [128, 2 * CHUNKS], f32, kind="ExternalOutput")

    E = n_pad
    Sin = mybir.ActivationFunctionType.Sin

    with TileContext(nc) as tc:
        with (
            tc.tile_pool(name="const", bufs=1) as cpool,
            tc.tile_pool(name="work", bufs=4) as pool,
            tc.tile_pool(name="cols", bufs=8) as colpool,
            tc.tile_pool(name="psum", bufs=1, space="PSUM") as psp,
        ):
            lhs = cpool.tile([7, CHUNKS * 128], bf16)
            rhsu = cpool.tile([6, 3 * E], bf16)
            rhsuc = cpool.tile([7, 3 * E], bf16)
            rhsd = cpool.tile([6, 2 * E], bf16)
            rhsg = cpool.tile([6, 3 * E], bf16)
            fout = cpool.tile([128, 2 * CHUNKS], f32)
            # matmul outputs must not straddle PSUM bank boundaries:
            # either the 3-slot arena fits one bank, or slots are 1KB-aligned
            assert 12 * E <= 2048 or E == 256, f"bad n_pad {E}"
            nc.sync.dma_start(lhs[:], lhs_d[:])
            nc.sync.dma_start(rhsu[:], rhsu_d[:])
            nc.sync.dma_start(rhsuc[:], rhsuc_d[:])
            nc.sync.dma_start(rhsd[:], rhsd_d[:])
            nc.sync.dma_start(rhsg[:], rhsg_d[:])

            # process chunks in pairs: PE matmuls stay per-chunk, but
            # FRAC/Sin/QR/recip/G run once per pair over strided rank-3 APs.
            # PSUM pair arenas are padded to 512-col halves so no matmul
            # output crosses a bank boundary.
            HB = 512  # psum half stride (cols)
            pairs = [
                [2 * p, 2 * p + 1] if 2 * p + 1 < CHUNKS else [2 * p]
                for p in range((CHUNKS + 1) // 2)
            ]
            for chunks in pairs:
                T = len(chunks)
                EB = 3 * E  # one chunk's block of 3 vertex planes

                # uu: [us(h0)|us(h1)|uc(h0)|uc(h1)] in 512-col halves (4 banks)
                uu = psp.tile([128, 4 * HB], f32, tag="uu")
                dd = psp.tile([128, 2 * HB], f32, tag="dd")
                gg = psp.tile([128, 2 * HB], f32, tag="gg")

                mm = nc.tensor.matmul
                for h, c in enumerate(chunks):
                    l6 = lhs[0:6, c * 128 : (c + 1) * 128]
                    l7 = lhs[:, c * 128 : (c + 1) * 128]
                    b = h * HB
                    # group by stationary operand: all l6 matmuls, then l7
                    for v in range(3):
                        mm(uu[:, b + v * E : b + (v + 1) * E], l6,
                           rhsu[:, v * E : (v + 1) * E], start=True, stop=True)
                    mm(dd[:, b : b + E], l6, rhsd[:, 0:E], start=True, stop=True)
                    mm(dd[:, b + E : b + 2 * E], l6, rhsd[:, E : 2 * E],
                       start=True, stop=True)
                    mm(gg[:, b : b + E], l6, rhsg[:, 0:E],
                       start=True, stop=True)
                    mm(gg[:, b + E : b + 2 * E], l6, rhsg[:, 2 * E : 3 * E],
                       start=True, stop=True)
                    for v in range(3):
                        mm(uu[:, 2 * HB + b + v * E : 2 * HB + b + (v + 1) * E],
                           l7, rhsuc[:, v * E : (v + 1) * E],
                           start=True, stop=True)

                def psum_blocks(ap, off, width):
                    """(128, nblk, width) view of used cols of a PSUM arena
                    whose data sits in T-strided 512-col halves."""
                    stride = HB if T == 2 else 2 * HB
                    return ap.rearrange("p (t x) -> p t x", x=stride)[
                        :, :, off : off + width
                    ]

                def blocks(ap, off, width, stride):
                    """(128, nblk, width) view of a compact tile."""
                    return ap.rearrange("p (t x) -> p t x", x=stride)[
                        :, :, off : off + width
                    ]

                # d12 PSUM->SBUF copy first: it gates the QR->recip->G chain,
                # while the Sin below only gates the final accumulation
                d12s = pool.tile([128, T * E], f32, tag="d12s")
                nc.scalar.activation(
                    blocks(d12s[:], 0, E, E), psum_blocks(dd[:], E, E),
                    mybir.ActivationFunctionType.Copy,
                )

                # FRAC: arg = 2*pi*(u - round(u)); sin blocks then cos blocks
                arg = pool.tile([128, 2 * T * EB], f32, tag="arg")
                cd = nc.vector._custom_dve
                cd(FRAC, out=blocks(arg[:], 0, EB, EB),
                   in0=psum_blocks(uu[:], 0, EB), s0=MAGIC, s1=TWO_PI)

                # trig: [s(h0)|s(h1)|c(h0)|c(h1)] chunk blocks of [v0|v1|v2]
                tr = pool.tile([128, 2 * T * EB], f32, tag="tr")
                nc.scalar.activation(tr[:], arg[:], Sin)
                mQ = pool.tile([128, T * E], f32, tag="mQ")
                cd(QR3, out=blocks(mQ[:], 0, E, E), in0=blocks(d12s[:], 0, E, E),
                   in1=psum_blocks(dd[:], 0, E))
                # 51-ULP reciprocal is plenty: overall error is dominated by
                # fp32 trig-argument rounding (verified vs fp64 reference)
                R = pool.tile([128, T * E], f32, tag="R")
                nc.vector.reciprocal_approx_fast(out=R[:], in_=mQ[:])

                # Gt layout: chunk-major [h][v] so the v-sum can fuse into
                # the accumulation pass; G1 = -(G0+G2) since d20 = -(d01+d12)
                Gt = pool.tile([128, T * EB], f32, tag="Gt")
                nc.vector.tensor_mul(
                    blocks(Gt[:], 0, E, EB), psum_blocks(gg[:], 0, E),
                    blocks(R[:], 0, E, E),
                )
                nc.vector.tensor_mul(
                    blocks(Gt[:], 2 * E, E, EB), psum_blocks(gg[:], E, E),
                    blocks(R[:], 0, E, E),
                )
                cd(NEG_ADD, out=blocks(Gt[:], E, E, EB),
                   in0=blocks(Gt[:], 0, E, EB), in1=blocks(Gt[:], 2 * E, E, EB))

                # one fused multiply+reduce per (chunk, component): the free
                # dim spans all 3 vertex planes, so accum does the v-sum too
                scr = pool.tile([128, EB], f32, tag="scr")
                for h, c in enumerate(chunks):
                    g = Gt[:, h * EB : (h + 1) * EB]
                    cd(MUL_ACC, out=scr[:], in0=g,
                       in1=tr[:, (T + h) * EB : (T + h + 1) * EB], s0=0.0,
                       accum_out=fout[:, 2 * c : 2 * c + 1])
                    cd(MUL_ACC, out=scr[:], in0=g,
                       in1=tr[:, h * EB : (h + 1) * EB], s0=0.0,
                       accum_out=fout[:, 2 * c + 1 : 2 * c + 2])

            nc.sync.dma_start(fout_d[:], fout[:])

    nc.compile()
    return nc


def _host_prep_group(P, Dagg, n_pad):
    """Build per-core input maps for one padded element group."""
    n_eff = P.shape[0]
    # pad with copies of element 0 carrying zero density (zero contribution)
    if n_pad > n_eff:
        P = np.concatenate([P, np.repeat(P[:1], n_pad - n_eff, axis=0)], axis=0)
        Dagg = np.concatenate(
            [Dagg, np.zeros((n_pad - n_eff, Dagg.shape[1]))], axis=0
        )
    ne = n_pad

    # CD = 2 * area * D via Cayley-Menger (matches reference up to fp rounding)
    D2 = ((P[:, :, None, :] - P[:, None, :, :]) ** 2).sum(-1)
    B = np.ones((ne, 4, 4))
    B[:, 0, 0] = 0.0
    B[:, 1:, 1:] = D2
    vol2 = (-1.0) / 4.0 * np.linalg.det(B) / 4.0  # ((-1)^3)/(2^2)/(2!^2)*det
    content = np.sqrt(np.clip(vol2, 0.0, None))
    CD = 2.0 * content[:, None] * Dagg  # (ne, n_ch=1)
    cd = CD[:, 0]  # n_ch == 1

    Px = P[:, :, 0]  # (ne, 3)
    Py = P[:, :, 1]
    dPx = Px - np.roll(Px, -1, axis=1)  # [d01, d12, d20] coefficients
    dPy = Py - np.roll(Py, -1, axis=1)

    def stack6(ax, ay):
        """rows [axh, axm, axl, ayh, aym, ayl] as bf16 (256 cols)."""
        xh, xm, xl = _split3(ax)
        yh, ym, yl = _split3(ay)
        return np.stack([xh, xm, xl, yh, ym, yl]).astype(ml_dtypes.bfloat16)

    rhsu = np.concatenate([stack6(Px[:, v], Py[:, v]) for v in range(3)], axis=1)
    quarter = np.full((1, ne), 0.25, ml_dtypes.bfloat16)
    rhsuc = np.concatenate(
        [
            np.concatenate([stack6(Px[:, v], Py[:, v]), quarter], axis=0)
            for v in range(3)
        ],
        axis=1,
    )
    rhsd = np.concatenate(
        [stack6(TWO_PI * dPx[:, k], TWO_PI * dPy[:, k]) for k in (0, 1)], axis=1
    )
    # gg_v pairs: c0<->d12, c1<->d20, c2<->d01
    pair = [1, 2, 0]
    rhsg = np.concatenate(
        [
            stack6(TWO_PI * cd * dPx[:, pair[v]], TWO_PI * cd * dPy[:, pair[v]])
            for v in range(3)
        ],
        axis=1,
    )

    kxv = np.fft.fftfreq(RES0, d=1.0 / RES0)  # row -> freq value
    in_maps = []
    for r in range(N_CORES):
        q = np.arange(CHUNKS * 128)
        lr = q // KYPAD
        kyi = q % KYPAD
        kxrow = kxv[32 * r + lr]
        lhs = np.zeros((7, CHUNKS * 128), np.float32)
        lhs[0:3] = kxrow
        lhs[3:6] = kyi
        lhs[6] = 1.0
        in_maps.append(
            {
                "lhs7": lhs.astype(ml_dtypes.bfloat16),
                "rhsu": rhsu,
                "rhsuc": rhsuc,
                "rhsd": rhsd,
                "rhsg": rhsg,
            }
        )
    return in_maps, float(np.sum(cd))


# largest element count whose 3-plane PSUM arena fits one 512-col half
_MAX_GROUP = 170


def kernel(V, E, D, _want_trace=False):
    from concourse.bass_utils import run_bass_kernel_spmd

    V = np.asarray(V, np.float32)
    E = np.asarray(E)
    D = np.asarray(D, np.float32)

    # identical elements (same vertex-index rows) contribute identical
    # spectra scaled by their D -> deduplicate and aggregate D
    Eu, inv = np.unique(E, axis=0, return_inverse=True)
    Dagg = np.zeros((Eu.shape[0], D.shape[1]), np.float64)
    np.add.at(Dagg, inv.reshape(-1), D.astype(np.float64))
    n_eff = Eu.shape[0]
    P = V[Eu].astype(np.float64)  # (n_eff, 3, 2)

    # split into groups small enough for the PSUM layout; partial spectra
    # are linear in elements, so group results just add
    n_groups = -(-n_eff // _MAX_GROUP)
    per = -(-n_eff // n_groups)
    n_pad = max(8, -(-per // 2) * 2)
    if n_pad not in _compiled:
        _compiled[n_pad] = _build_program(n_pad)
    nc = _compiled[n_pad]

    fo_sum = [np.zeros((128, 2 * CHUNKS), np.float64) for _ in range(N_CORES)]
    cd_total = 0.0
    res = None
    for g in range(n_groups):
        sl = slice(g * per, min((g + 1) * per, n_eff))
        in_maps, cd_sum = _host_prep_group(P[sl], Dagg[sl], n_pad)
        cd_total += cd_sum
        res = run_bass_kernel_spmd(
            nc, in_maps, core_ids=list(range(N_CORES)), trace=_want_trace
        )
        for r in range(N_CORES):
            fo_sum[r] += res.results[r]["fout"]

    F = np.zeros((RES0, RES1, 1, 2), np.float32)
    for r in range(N_CORES):
        fo = fo_sum[r].astype(np.float32)  # (128, 2*CHUNKS)
        re_raw = fo[:, 0::2].T.reshape(-1)  # (33*128,) chunk-major
        im_raw = fo[:, 1::2].T.reshape(-1)
        re = re_raw.reshape(ROWS_PER_CORE, KYPAD)[:, :RES1]
        im = im_raw.reshape(ROWS_PER_CORE, KYPAD)[:, :RES1]
        F[32 * r : 32 * r + 32, :, 0, 0] = -65536.0 * re
        F[32 * r : 32 * r + 32, :, 0, 1] = 65536.0 * im
    F[0, 0, 0, :] = np.float32(32768.0 * cd_total)
    if _want_trace:
        return F, res
    return F
